# revision 25
# baseline (speedup 1.0000x reference)
"""Trainium2 Bass kernel for nn_DiscreteGraphLearning (8 NeuronCores).

Strategy:
  - Encoder is sharded over TIME (the fc contraction dim): each core owns
    conv2-output times [752k, 752k+752) -> fc_w is read once total.
  - conv1/conv2 as packed matmuls: nodes on the moving free dim (307),
    (time-offset x channel) on partitions.
  - BatchNorm stats via per-tile bn_stats + masked combine; global stats via
    three small AllReduces; bn affine folded into the next layer's stationary
    weights / biases (no normalization pass over the data).
  - Gumbel-softmax straight-through == hard argmax: adj[i,j] =
    (logit0+g0 >= logit1+g1); only a margin sign is computed on device.
  - Edge phase row-sharded: core k computes adj rows [39k, 39k+39).
"""
import sys, os, types
import numpy as np

N = 307
ED = 96
T2 = 5982          # conv2 output times (real)
T1 = 5991          # conv1 output times (real)
NCORES = 8
L2C = 752          # conv2-output times per core (47 blocks of 16)
NB2 = 47           # conv2 16-blocks per core
NT2 = 94           # y2 psum tiles per core (8 times each)
NB1 = 48           # y1 16-blocks per core
NXT = 6            # xA tiles (and 6 xB tiles)
XROWS = 800        # x rows per core (6*128 + ... covers 777 needed)
RPC = 39           # adj rows per core slice (8*39=312 >= 307)
EPS = 1e-5
NTOT1 = float(T1 * N)
NTOT2 = float(T2 * N)
CE, CO = 154.0, 153.0   # bn_stats even/odd counts for 307-wide tiles
R1 = (2, 6, 10, 14)                    # y1 pairs stats-routed to ACT
R2 = (2, 9, 16, 23, 30, 37)            # y2 pairs stats-routed to ACT


def _install_ntff_hook():
    if "antenv.axon_hooks" in sys.modules:
        return
    try:
        mod = types.ModuleType("antenv.axon_hooks")
        mod._hook = None
        def set_axon_ntff_profile_hook(h): mod._hook = h
        def get_axon_ntff_profile_hook(): return mod._hook
        mod.set_axon_ntff_profile_hook = set_axon_ntff_profile_hook
        mod.get_axon_ntff_profile_hook = get_axon_ntff_profile_hook
        sys.modules["antenv.axon_hooks"] = mod
        import antenv
        antenv.axon_hooks = mod
        from trn_agent_boot.trn_boot import _ntff_profile_via_ctypes
        mod.set_axon_ntff_profile_hook(
            _ntff_profile_via_ctypes("/opt/axon/libaxon_pjrt.so"))
    except Exception:
        pass


_install_ntff_hook()

import concourse.bass as bass
import concourse.bacc as bacc
import concourse.tile as tile
from concourse import mybir, bass_utils

F32 = mybir.dt.float32
F16 = mybir.dt.float16
AF = mybir.ActivationFunctionType
OP = mybir.AluOpType


# ---------------------------------------------------------------- device program
def _build_program():
    nc = bacc.Bacc("TRN2", target_bir_lowering=False, debug=False,
                   num_devices=NCORES)

    def din(name, shape):
        return nc.dram_tensor(name, list(shape), F32, kind="ExternalInput").ap()

    def din16(name, shape):
        return nc.dram_tensor(name, list(shape), F16, kind="ExternalInput").ap()

    xc = din16("xc", (NB1, 75, N))
    fcwh = din16("fcwh", (NT2, 128, ED))
    fcwr = din16("fcwr", (NT2, 128, ED))
    s1cd = din16("s1c", (75, 128))
    NPACK = 512 + 16 + 96 + 8 + 16 + 128 + 128 + 1 + 6 + 96 + 96 + 1 + 2 + 128 + 48 + 94 + 3 * RPC + 2 * N
    packd = nc.dram_tensor("pack", [128, NPACK], F32, kind="ExternalInput").ap()
    adj = nc.dram_tensor("adj", [RPC, N], F32, kind="ExternalOutput").ap()

    rg = [list(range(NCORES))]

    with tile.TileContext(nc) as tc:
        with (tc.tile_pool(name="P", bufs=1) as P,
              tc.tile_pool(name="Pdram", bufs=1, space="DRAM") as PD,
              tc.tile_pool(name="Psmall", bufs=2, space="PSUM") as PS):

            # ---------------- persistent small sbuf tensors
            def sload(ap, shape, tag):
                t = P.tile(list(shape), F32, tag=tag)
                nc.sync.dma_start(t[:], ap)
                return t
            def sload16(ap, shape, tag):
                t = P.tile(list(shape), F16, tag=tag)
                nc.sync.dma_start(t[:], ap)
                return t
            s1c = sload16(s1cd, (75, 128), "s1c")
            pk = P.tile([128, NPACK], F32, tag="pk")
            nc.sync.dma_start(pk[:], packd)
            off = [0]
            def pview(pn, w):
                o = off[0]; off[0] += w
                return pk[0:pn, o:o + w]
            s2 = pview(128, 512).rearrange("p (a c) -> p a c", a=4)
            w2s = pview(9, 16)
            wbar = pview(17, ED)
            sel8 = pview(128, 8)
            sel16 = pview(128, 16)
            rp8 = pview(8, 128)
            rp16 = pview(16, 128)
            b1v = pview(128, 1)
            g1 = pview(8, 1); bb1 = pview(8, 1)
            g2 = pview(16, 1); bb2 = pview(16, 1)
            g3 = pview(ED, 1); bb3 = pview(ED, 1)
            fow_a = pview(ED, ED)
            fow_b = pview(ED, ED)
            fob_s = pview(ED, 1)
            fcat_s = pview(97, 2)
            ident = pview(128, 128)
            m1s = pview(128, NB1)
            m2s = pview(128, NT2)
            rsel_s = pview(128, 3 * RPC).rearrange("p (a c) -> p a c", a=3)
            u0s = pview(128, N)
            u1s = pview(128, N)

            stats1 = P.tile([128, NB1 * 6], F32, tag="stats1")
            stats2 = P.tile([128, NT2 * 6], F32, tag="stats2")
            asum1 = P.tile([128, len(R1)], F32, tag="asum1")
            asq1 = P.tile([128, len(R1)], F32, tag="asq1")
            asum2 = P.tile([128, len(R2)], F32, tag="asum2")
            asq2 = P.tile([128, len(R2)], F32, tag="asq2")

            # ============ helper: masked (sum, sumsq) from bn_stats buffer
            # stats: [128, nt*6], mask: [128, nt] -> out2 [128,2] (sum, sumsq)
            def masked_sums(stats, mask, nt, pfx):
                me = stats[:, 1::6]
                mo = stats[:, 4::6]
                cve = stats[:, 2::6]
                cvo = stats[:, 5::6]
                ta = P.tile([128, nt], F32, tag=pfx + "ta")
                tb = P.tile([128, nt], F32, tag=pfx + "tb")
                out2 = P.tile([128, 2], F32, tag=pfx + "o2")
                # sum: (154*me + 153*mo) * mask
                nc.vector.tensor_scalar(ta[:], mo, CO, None, OP.mult)
                nc.vector.scalar_tensor_tensor(ta[:], me, CE, ta[:], OP.mult, OP.add)
                nc.vector.tensor_tensor(ta[:], ta[:], mask, OP.mult)
                nc.vector.reduce_sum(out2[:, 0:1], ta[:], axis=mybir.AxisListType.X)
                asum, asq = (asum1, asq1) if nt == NB1 else (asum2, asq2)
                hs = P.tile([128, 1], F32, tag=pfx + "hs")
                nc.vector.reduce_sum(hs[:], asum[:], axis=mybir.AxisListType.X)
                nc.vector.tensor_add(out2[:, 0:1], out2[:, 0:1], hs[:])
                # sumsq: (cve + 154*me^2 + cvo + 153*mo^2) * mask
                nc.vector.tensor_tensor(ta[:], me, me, OP.mult)
                nc.vector.scalar_tensor_tensor(ta[:], ta[:], CE, cve, OP.mult, OP.add)
                nc.vector.tensor_tensor(tb[:], mo, mo, OP.mult)
                nc.vector.scalar_tensor_tensor(tb[:], tb[:], CO, cvo, OP.mult, OP.add)
                nc.vector.tensor_add(ta[:], ta[:], tb[:])
                nc.vector.tensor_tensor(ta[:], ta[:], mask, OP.mult)
                nc.vector.reduce_sum(out2[:, 1:2], ta[:], axis=mybir.AxisListType.X)
                nc.vector.reduce_sum(hs[:], asq[:], axis=mybir.AxisListType.X)
                nc.vector.tensor_add(out2[:, 1:2], out2[:, 1:2], hs[:])
                return out2

            # ============ helper: stats -> (a, d) affine coeffs
            # st [C,2] global (sum,sumsq); returns a [C,1], d [C,1]
            def bn_coeffs(stg, Cn, ntot, gam, bet, pfx):
                mC = P.tile([Cn, 1], F32, tag=pfx + "m")
                vC = P.tile([Cn, 1], F32, tag=pfx + "v")
                aC = P.tile([Cn, 1], F32, tag=pfx + "a")
                dC = P.tile([Cn, 1], F32, tag=pfx + "d")
                h = P.tile([Cn, 1], F32, tag=pfx + "h")
                nc.vector.tensor_scalar(mC[:], stg[:, 0:1], 1.0 / ntot, None, OP.mult)
                nc.vector.tensor_scalar(vC[:], stg[:, 1:2], 1.0 / ntot, None, OP.mult)
                nc.vector.tensor_tensor(h[:], mC[:], mC[:], OP.mult)
                nc.vector.tensor_sub(vC[:], vC[:], h[:])          # var
                nc.vector.tensor_scalar(vC[:], vC[:], EPS, None, OP.add)  # var+eps
                nc.vector.reciprocal(aC[:], vC[:])
                nc.scalar.activation(aC[:], aC[:], AF.Sqrt)
                for _ in range(2):  # Newton: r <- r*(1.5 - 0.5*v*r^2)
                    nc.vector.tensor_tensor(h[:], aC[:], aC[:], OP.mult)
                    nc.vector.tensor_tensor(h[:], h[:], vC[:], OP.mult)
                    nc.vector.tensor_scalar(h[:], h[:], -0.5, 1.5, OP.mult, OP.add)
                    nc.vector.tensor_tensor(aC[:], aC[:], h[:], OP.mult)
                nc.vector.tensor_tensor(aC[:], aC[:], gam, OP.mult)  # a = g*inv
                nc.vector.tensor_tensor(h[:], aC[:], mC[:], OP.mult)
                nc.vector.tensor_sub(dC[:], bet, h[:])         # d = b - a*m
                return aC, dC

            # ============ helper: AllReduce a small sbuf tensor
            def allreduce(sb, shape, pfx):
                cin = PD.tile(list(shape), F32, tag=pfx + "ci")
                cout = PD.tile(list(shape), F32, tag=pfx + "co")
                nc.sync.dma_start(cin[:], sb[:])
                nc.gpsimd.collective_compute(
                    "AllReduce", OP.add, replica_groups=rg,
                    ins=[cin.opt()], outs=[cout.opt()])
                og = P.tile(list(shape), F32, tag=pfx + "og")
                nc.sync.dma_start(og[:], cout[:])
                return og

            # ============ helper: AllGather small stats [Cn,2] + local sum
            def allgather_sum(sb, Cn, pfx):
                cin = PD.tile([Cn, 2], F32, tag=pfx + "ci")
                cout = PD.tile([Cn * NCORES, 2], F32, tag=pfx + "co")
                nc.sync.dma_start(cin[:], sb[:])
                nc.gpsimd.collective_compute(
                    "AllGather", OP.bypass, replica_groups=rg,
                    ins=[cin.opt()], outs=[cout.opt()])
                # load back as [Cn, (2 vals, 8 cores)] then reduce over cores
                og = P.tile([Cn, 2, NCORES], F32, tag=pfx + "og")
                nc.sync.dma_start(
                    og[:], cout[:].rearrange("(k c) v -> c v k", k=NCORES))
                red = P.tile([Cn, 2], F32, tag=pfx + "rd")
                nc.vector.reduce_sum(red[:], og[:], axis=mybir.AxisListType.X)
                return red

            # ============ helper: replicate [C,1] -> [128,1] via pattern MM
            def replicate(vecC, pat, Cn, pfx):
                ps = PS.tile([128, 1], F32, tag="ps")
                nc.tensor.matmul(ps[:], pat, vecC[:], start=True, stop=True)
                ov = P.tile([128, 1], F32, tag=pfx + "rv")
                nc.vector.tensor_copy(ov[:], ps[:])
                return ov

            # ================ phase A: conv1
            y1tiles = []
            PY2_cm = tc.tile_pool(name="Py2", bufs=NB2)
            PY2 = PY2_cm.__enter__()
            with (tc.tile_pool(name="Py1", bufs=24) as PY1,
                  tc.tile_pool(name="Px", bufs=10) as PX,
                  tc.tile_pool(name="Pdmp", bufs=3) as PDMP,
                  tc.tile_pool(name="Pq", bufs=3, space="PSUM") as PQ):
                for m in range(24):
                    xt_ = PX.tile([75, 2, N], F16, tag="xc")
                    eng = nc.sync if m % 2 == 0 else nc.gpsimd
                    eng.dma_start(xt_[:], xc[2 * m:2 * m + 2]
                                  .rearrange("b p n -> p b n"))
                    pr = PQ.tile([128, 2, 512], F32, tag="q")
                    for j in range(2):
                        nc.tensor.matmul(pr[:, j, 0:N], s1c[:],
                                         xt_[:, j, :],
                                         start=True, stop=True)
                    yp = PY1.tile([128, 2, N], F16, tag="y1p")
                    y1tiles.append(yp)
                    if m in R1:
                        ri = R1.index(m)
                        nc.scalar.activation(yp[:], pr[:, :, 0:N], AF.Relu,
                                             bias=b1v, scale=1.0,
                                             accum_out=asum1[:, ri:ri + 1])
                        dmp = PDMP.tile([128, 2, N], F32, tag="dmp")
                        nc.scalar.activation(
                            dmp[:], yp[:], AF.Square,
                            accum_out=asq1[:, ri:ri + 1])
                    else:
                        nc.scalar.activation(yp[:], pr[:, :, 0:N], AF.Relu,
                                             bias=b1v, scale=1.0)
                        for j in range(2):
                            b = 2 * m + j
                            nc.vector.bn_stats(stats1[:, 6 * b:6 * b + 6],
                                               yp[:, j, :])
                y1blk = {b: (y1tiles[b // 2], b % 2) for b in range(NB1)}

                # ---------------- AR1: bn1 coeffs
                s1m = masked_sums(stats1, m1s, NB1, "s1_")
                pc8 = PS.tile([8, 2], F32, tag="ps")
                nc.tensor.matmul(pc8[:], sel8, s1m[:], start=True, stop=True)
                sb8 = P.tile([8, 2], F32, tag="sb8")
                nc.vector.tensor_copy(sb8[:], pc8[:])
                st1g = allgather_sum(sb8, 8, "ar1")
                a1, d1 = bn_coeffs(st1g, 8, NTOT1, g1, bb1, "c1_")
                a1v = replicate(a1, rp8, 8, "a1")
                # scale conv2 stationaries (fp32), then split to fp16 h+r
                s2sc = P.tile([128, 4, 128], F32, tag="s2sc")
                nc.vector.tensor_scalar(s2sc[:], s2, a1v[:], None, OP.mult)
                s2h = P.tile([128, 4, 128], F16, tag="s2h")
                nc.vector.tensor_copy(s2h[:], s2sc[:])
                s2r = P.tile([128, 4, 128], F16, tag="s2r")
                nc.vector.tensor_sub(s2r[:], s2sc[:], s2h[:])
                # conv2 bias b2eff = W2s·d1 + conv2_b  (w2s row 8 = conv2_b)
                d1e = P.tile([9, 1], F32, tag="d1e")
                nc.vector.memset(d1e[:], 1.0)
                nc.vector.tensor_copy(d1e[0:8, :], d1[:])
                pb2 = PS.tile([16, 1], F32, tag="ps")
                nc.tensor.matmul(pb2[:], w2s, d1e[:], start=True, stop=True)
                b2c = P.tile([16, 1], F32, tag="b2c")
                nc.vector.tensor_copy(b2c[:], pb2[:])
                b2v = replicate(b2c, rp16, 16, "b2")

                # ================ phase B: conv2
                SA1h, SA1r = s2h[:, 0, :], s2r[:, 0, :]
                SB1 = s2h[:, 1, :]
                SA2h, SA2r = s2h[:, 2, :], s2r[:, 2, :]
                SB2 = s2h[:, 3, :]
                y2tiles = []
                for B in range(NB2):
                    ta, sa = y1blk[B]
                    tb, sb = y1blk[B + 1]
                    y1a = ta[:, sa, :]
                    y1b = tb[:, sb, :]
                    pq = PQ.tile([128, 2, 512], F32, tag="q")
                    nc.tensor.matmul(pq[:, 0, 0:N], SA1h, y1a, start=True, stop=False)
                    nc.tensor.matmul(pq[:, 0, 0:N], SA1r, y1a, start=False, stop=False)
                    nc.tensor.matmul(pq[:, 0, 0:N], SB1, y1b, start=False, stop=True)
                    nc.tensor.matmul(pq[:, 1, 0:N], SA2h, y1a, start=True, stop=False)
                    nc.tensor.matmul(pq[:, 1, 0:N], SA2r, y1a, start=False, stop=False)
                    nc.tensor.matmul(pq[:, 1, 0:N], SB2, y1b, start=False, stop=True)
                    yq = PY2.tile([128, 2, N], F32, tag="y2p")
                    y2tiles.append(yq)
                    if B in R2:
                        ri = R2.index(B)
                        nc.scalar.activation(yq[:], pq[:, :, 0:N], AF.Relu,
                                             bias=b2v[:], scale=1.0,
                                             accum_out=asum2[:, ri:ri + 1])
                        dmp = PDMP.tile([128, 2, N], F32, tag="dmp")
                        nc.scalar.activation(
                            dmp[:], yq[:], AF.Square,
                            accum_out=asq2[:, ri:ri + 1])
                    else:
                        nc.scalar.activation(yq[:], pq[:, :, 0:N], AF.Relu,
                                             bias=b2v[:], scale=1.0)
                        for j in range(2):
                            jj = 2 * B + j
                            nc.vector.bn_stats(stats2[:, 6 * jj:6 * jj + 6],
                                               yq[:, j, :])

            # ---------------- AR2: bn2 coeffs
            s2m = masked_sums(stats2, m2s, NT2, "s2_")
            pc16 = PS.tile([16, 2], F32, tag="ps")
            nc.tensor.matmul(pc16[:], sel16, s2m[:], start=True, stop=True)
            sb16 = P.tile([16, 2], F32, tag="sb16")
            nc.vector.tensor_copy(sb16[:], pc16[:])
            st2g = allgather_sum(sb16, 16, "ar2")
            a2, d2 = bn_coeffs(st2g, 16, NTOT2, g2, bb2, "c2_")
            a2v = replicate(a2, rp16, 16, "a2")
            # fc bias = Wbar·d2 + fc_b  (wbar row 16 = fc_b)
            d2e = P.tile([17, 1], F32, tag="d2e")
            nc.vector.memset(d2e[:], 1.0)
            nc.vector.tensor_copy(d2e[0:16, :], d2[:])
            pfb = PS.tile([ED, 1], F32, tag="ps")
            nc.tensor.matmul(pfb[:], wbar, d2e[:], start=True, stop=True)
            fcb = P.tile([ED, 1], F32, tag="fcb")
            nc.vector.tensor_copy(fcb[:], pfb[:])

            # ================ phase C: fc
            with (tc.tile_pool(name="Py2f", bufs=6) as PY2F,
                  tc.tile_pool(name="Pfw", bufs=12) as PF,
                  tc.tile_pool(name="Pfc", bufs=2, space="PSUM") as PFC):
                psfc = PFC.tile([ED, N], F32, tag="fcacc")
                psfc2 = PFC.tile([ED, N], F32, tag="fcacc")
                NCH = 12  # ceil(94/8)
                fwch = []
                for cch in range(NCH):
                    j0 = 8 * cch
                    nj = min(8, NT2 - j0)
                    fh = PF.tile([128, 8, ED], F16, tag="fh")
                    nc.gpsimd.dma_start(fh[:, 0:nj, :],
                                        fcwh[j0:j0 + nj].rearrange("j p c -> p j c"))
                    fr = PF.tile([128, 8, ED], F16, tag="fr")
                    nc.gpsimd.dma_start(fr[:, 0:nj, :],
                                        fcwr[j0:j0 + nj].rearrange("j p c -> p j c"))
                    fwch.append((fh, fr))
                for p_ in range(NB2):
                    yf = PY2F.tile([128, 2, N], F16, tag="y2f")
                    if p_ % 8 < 5:
                        nc.vector.tensor_scalar(yf[:], y2tiles[p_][:],
                                                a2v[:], None, OP.mult)
                    else:
                        nc.scalar.activation(yf[:], y2tiles[p_][:],
                                             AF.Identity, bias=0.0,
                                             scale=a2v[:])
                    for j in (2 * p_, 2 * p_ + 1):
                        fh, fr = fwch[j // 8]
                        mv = yf[:, j % 2, :]
                        acc = psfc if j % 2 == 0 else psfc2
                        nc.tensor.matmul(acc[:], fh[:, j % 8, :], mv,
                                         start=(j < 2), stop=False)
                        nc.tensor.matmul(acc[:], fr[:, j % 8, :], mv,
                                         start=False, stop=(j >= NT2 - 2))
                zpart = P.tile([ED, N], F32, tag="zpart")
                nc.vector.tensor_copy(zpart[:], psfc[:])
                nc.vector.tensor_tensor(zpart[:], zpart[:], psfc2[:], OP.add)
            PY2_cm.__exit__(None, None, None)

            # ---------------- AR3: fc partial sums (gumbel overlaps the wait)
            cin3 = PD.tile([ED, N], F32, tag="ar3ci")
            cout3 = PD.tile([ED, N], F32, tag="ar3co")
            nc.sync.dma_start(cin3[:], zpart[:])
            nc.gpsimd.collective_compute(
                "AllReduce", OP.add, replica_groups=rg,
                ins=[cin3.opt()], outs=[cout3.opt()])
            # gumbel: g = L1 - L0, Lk = log(-log(uk+e)+e)
            e20 = P.tile([128, 1], F32, tag="e20")
            nc.vector.memset(e20[:], 1e-20)
            gt = P.tile([128, N], F32, tag="gt")
            l0 = P.tile([128, N], F32, tag="l0")
            eb = e20[:]
            nc.scalar.activation(l0[:], u0s, AF.Ln, bias=eb, scale=1.0)
            nc.scalar.activation(l0[:], l0[:], AF.Ln, bias=eb, scale=-1.0)
            nc.scalar.activation(gt[:], u1s, AF.Ln, bias=eb, scale=1.0)
            nc.scalar.activation(gt[:], gt[:], AF.Ln, bias=eb, scale=-1.0)
            nc.vector.tensor_sub(gt[:], gt[:], l0[:])

            zfull = P.tile([ED, N], F32, tag="ar3og")
            nc.sync.dma_start(zfull[:], cout3[:])

            # ================ phase D: bn3 + edge phase
            nf0 = P.tile([ED, N], F32, tag="nf0")
            nc.scalar.activation(nf0[:], zfull[:], AF.Relu, bias=fcb[:], scale=1.0)
            st3 = P.tile([ED, 6], F32, tag="st3")
            nc.vector.bn_stats(st3[:], nf0[:])
            # global (over nodes) mean/var from even/odd stats
            s3 = P.tile([ED, 2], F32, tag="s3")
            h3 = P.tile([ED, 1], F32, tag="h3")
            nc.vector.tensor_scalar(s3[:, 0:1], st3[:, 4:5], CO, None, OP.mult)
            nc.vector.scalar_tensor_tensor(s3[:, 0:1], st3[:, 1:2], CE, s3[:, 0:1],
                                           OP.mult, OP.add)
            nc.vector.tensor_tensor(h3[:], st3[:, 1:2], st3[:, 1:2], OP.mult)
            nc.vector.scalar_tensor_tensor(s3[:, 1:2], h3[:], CE, st3[:, 2:3],
                                           OP.mult, OP.add)
            nc.vector.tensor_tensor(h3[:], st3[:, 4:5], st3[:, 4:5], OP.mult)
            nc.vector.tensor_scalar(h3[:], h3[:], CO, None, OP.mult)
            nc.vector.tensor_add(s3[:, 1:2], s3[:, 1:2], h3[:])
            nc.vector.tensor_add(s3[:, 1:2], s3[:, 1:2], st3[:, 5:6])
            a3, d3 = bn_coeffs(s3, ED, float(N), g3, bb3, "c3_")
            nf = P.tile([ED, N], F32, tag="nf")
            nc.vector.tensor_scalar(nf[:], nf0[:], a3[:], d3[:], OP.mult, OP.add)

            # s_proj / r_proj
            psp = PS.tile([ED, N], F32, tag="ps")
            nc.tensor.matmul(psp[:], fow_a, nf[:], start=True, stop=True)
            sp = P.tile([ED, N], F32, tag="sp")
            nc.vector.tensor_copy(sp[:], psp[:])
            prp = PS.tile([ED, N], F32, tag="ps")
            nc.tensor.matmul(prp[:], fow_b, nf[:], start=True, stop=True)
            rp = P.tile([ED, N], F32, tag="rp")
            nc.vector.tensor_copy(rp[:], prp[:])

            # rp_core[:, il] = rp[:, 39k+il] + fob  via transpose + rsel
            rpc = P.tile([ED, RPC], F32, tag="rpc")
            pacc = PS.tile([ED, RPC], F32, tag="ps")
            for ti in range(3):
                cw = min(128, N - 128 * ti)
                ptr = PS.tile([128, ED], F32, tag="ps")
                nc.tensor.transpose(ptr[0:cw, :], rp[:, 128 * ti:128 * ti + cw],
                                    ident[0:ED, 0:ED])
                rpt = P.tile([128, ED], F32, tag="rpt")
                if cw < 128:
                    nc.vector.memset(rpt[:], 0.0)
                nc.vector.tensor_copy(rpt[0:cw, :], ptr[0:cw, :])
                nc.tensor.matmul(pacc[:], rpt[:], rsel_s[:, ti, :],
                                 start=(ti == 0), stop=(ti == 2))
            nc.scalar.activation(rpc[:], pacc[:], AF.Identity, bias=fob_s, scale=1.0)

            # wd_ext = fcat[:,0] - fcat[:,1]  ([97,1]; row 96 = bd)
            wbuf = P.tile([97, 63], F32, tag="wbuf")
            nc.vector.memset(wbuf[:], 0.0)
            nc.vector.tensor_sub(wbuf[:, 31:32], fcat_s[:, 0:1], fcat_s[:, 1:2])

            # edge rows (strip layout: adj row 10*s+q lives at partition 32*s+q)
            zz = P.tile([ED, N], F32, tag="zz")
            nc.vector.memset(zz[:], 0.0)
            psm = PS.tile([128, N], F32, tag="ps")
            ef_tiles = []
            for t_ in range(6):
                eft = P.tile([97, N], F32, tag=f"ef{t_}")
                nc.vector.memset(eft[96:97, :], 1.0)
                ef_tiles.append(eft)
            if True:
                order = [10 * s + q for q in range(10) for s in range(4)
                         if 10 * s + q < RPC]
                for idx, il in enumerate(order):
                    s, q = il // 10, il % 10
                    ef = ef_tiles[idx % 6]
                    if il % 2 == 0:
                        nc.scalar.activation(ef[0:ED, :], sp[:], AF.Relu,
                                             bias=rpc[:, il:il + 1], scale=1.0)
                    else:
                        nc.vector.scalar_tensor_tensor(
                            ef[0:ED, :], sp[:], rpc[:, il:il + 1], zz[:],
                            OP.add, OP.max)
                    nc.tensor.matmul(
                        psm[32 * s:32 * (s + 1), :],
                        wbuf[:, 31 - q:63 - q],
                        ef[:],
                        start=(q == 0), stop=(q == 9 or il == RPC - 1),
                        tile_position=(0, 32 * s), skip_group_check=True)

            dt = P.tile([128, N], F32, tag="dt")
            nc.vector.tensor_tensor(dt[:], psm[:], gt[:], OP.add)
            av = P.tile([128, N], F32, tag="av")
            nc.vector.tensor_scalar(av[:], dt[:], 0.0, None, OP.is_ge)
            for s in range(4):
                nrow = min(10, RPC - 10 * s)
                nc.sync.dma_start(adj[10 * s:10 * s + nrow, :],
                                  av[32 * s:32 * s + nrow, :])

    nc.compile()
    return nc


# ---------------------------------------------------------------- host prep
def _host_prep(inputs):
    nf_ = np.asarray(inputs["node_feats"], np.float32)        # [6000, 307]
    w1 = np.asarray(inputs["conv1_w"], np.float32)            # [8,1,10]
    b1 = np.asarray(inputs["conv1_b"], np.float32)
    w2 = np.asarray(inputs["conv2_w"], np.float32)            # [16,8,10]
    b2 = np.asarray(inputs["conv2_b"], np.float32)
    fc_w = np.asarray(inputs["fc_w"], np.float32)             # [95712,96]
    fc_b = np.asarray(inputs["fc_b"], np.float32)
    fow = np.asarray(inputs["fc_out_w"], np.float32)          # [192,96]
    fob = np.asarray(inputs["fc_out_b"], np.float32)
    fcat_w = np.asarray(inputs["fc_cat_w"], np.float32)       # [96,2]
    fcat_b = np.asarray(inputs["fc_cat_b"], np.float32)       # [2]
    uni = np.asarray(inputs["uniform"], np.float32)           # [307,307,2]

    # x padded [6064+, 307]
    xpad = np.zeros((L2C * 7 + XROWS, N), np.float32)
    xpad[:6000] = nf_

    # S1 [25,128] replicated at 4 partition offsets -> [128,128]
    S1 = np.zeros((25, 128), np.float32)
    for g in range(16):
        for c in range(8):
            for k in range(10):
                S1[g + k, g * 8 + c] = w1[c, 0, k]
    S1h = S1.astype(np.float16)
    S1r = (S1 - S1h.astype(np.float32)).astype(np.float16)
    s1c = np.concatenate([S1h, S1h, S1r], axis=0)  # [75, 128]

    # conv2 stationaries [4,128,128]
    s2 = np.zeros((4, 128, 128), np.float32)
    for g in range(16):
        for c in range(8):
            for t in range(8):
                for cc in range(16):
                    pin, pout = g * 8 + c, t * 16 + cc
                    k = g - t
                    if 0 <= k <= 9:
                        s2[0, pin, pout] = w2[cc, c, k]
                    if g == 0 and t == 7:
                        s2[1, pin, pout] = w2[cc, c, 9]
                    k = g - 8 - t
                    if 0 <= k <= 9:
                        s2[2, pin, pout] = w2[cc, c, k]
                    k = g + 8 - t
                    if 0 <= k <= 9:
                        s2[3, pin, pout] = w2[cc, c, k]

    w2se = np.zeros((9, 16), np.float32)
    w2se[0:8] = w2.sum(axis=2).T          # [c, c']
    w2se[8] = b2

    fc3 = fc_w.reshape(16, T2, ED)
    wbare = np.zeros((17, ED), np.float32)
    wbare[0:16] = fc3.sum(axis=1)
    wbare[16] = fc_b

    fcpad = np.zeros((16, L2C * 8, ED), np.float32)
    fcpad[:, :T2] = fc3

    p = np.arange(128)
    selc8 = (p[:, None] % 8 == np.arange(8)[None]).astype(np.float32)
    selc16 = (p[:, None] % 16 == np.arange(16)[None]).astype(np.float32)
    rep8 = selc8.T.copy()
    rep16 = selc16.T.copy()
    b1v = b1[p % 8].reshape(128, 1).astype(np.float32)

    fcat = np.zeros((97, 2), np.float32)
    fcat[0:96] = fcat_w
    fcat[96] = fcat_b

    def blk(a, pn, w):
        out = np.zeros((128, w), np.float32)
        out[:pn] = a.reshape(pn, w)
        return out

    pack_common = [
        blk(s2.transpose(1, 0, 2).reshape(128, 512), 128, 512),
        blk(w2se, 9, 16), blk(wbare, 17, ED),
        selc8, selc16, blk(rep8, 8, 128), blk(rep16, 16, 128), b1v,
        blk(np.asarray(inputs["bn1_g"], np.float32), 8, 1),
        blk(np.asarray(inputs["bn1_b"], np.float32), 8, 1),
        blk(np.asarray(inputs["bn2_g"], np.float32), 16, 1),
        blk(np.asarray(inputs["bn2_b"], np.float32), 16, 1),
        blk(np.asarray(inputs["bn3_g"], np.float32), ED, 1),
        blk(np.asarray(inputs["bn3_b"], np.float32), ED, 1),
        blk(fow[0:ED], ED, ED), blk(fow[ED:192], ED, ED),
        blk(fob, ED, 1), blk(fcat, 97, 2),
        np.eye(128, dtype=np.float32),
    ]

    in_maps = []
    for k in range(NCORES):
        t0 = L2C * k
        xcf = np.zeros((NB1, 25, N), np.float32)
        for b in range(NB1):
            xcf[b] = xpad[t0 + 16 * b: t0 + 16 * b + 25]
        xh_ = xcf.astype(np.float16)
        xr_ = (xcf - xh_.astype(np.float32)).astype(np.float16)
        xck = np.concatenate([xh_, xr_, xh_], axis=1)  # [48, 75, N]
        fcwk = fcpad[:, t0:t0 + L2C].reshape(16, NT2, 8, ED) \
            .transpose(1, 2, 0, 3).reshape(NT2, 128, ED)
        fcwkh = fcwk.astype(np.float16)
        fcwkr = (fcwk - fcwkh.astype(np.float32)).astype(np.float16)
        # masks
        bidx = np.arange(NB1)
        g_of_p = p // 8
        t1glob = t0 + 16 * bidx[None, :] + g_of_p[:, None]
        m1 = ((bidx[None, :] < 47) & (t1glob < T1)).astype(np.float32)
        for m_ in (2, 6, 10, 14):          # R1 pairs -> blocks 2m, 2m+1
            m1[:, 2 * m_] = 0.0
            m1[:, 2 * m_ + 1] = 0.0
        jidx = np.arange(NT2)
        t_of_p = p // 16
        t2glob = t0 + 8 * jidx[None, :] + t_of_p[:, None]
        m2 = (t2glob < T2).astype(np.float32)
        for B_ in (2, 9, 16, 23, 30, 37):   # R2 pairs -> tiles 2B, 2B+1
            m2[:, 2 * B_] = 0.0
            m2[:, 2 * B_ + 1] = 0.0
        # edge rows
        rows = np.minimum(np.arange(RPC * k, RPC * k + RPC), N - 1)
        rsel = np.zeros((3, 128, RPC), np.float32)
        for il, r in enumerate(rows):
            rsel[r // 128, r % 128, il] = 1.0
        u0 = np.full((128, N), 0.5, np.float32)
        u1 = np.full((128, N), 0.5, np.float32)
        for il, r in enumerate(rows):
            strip, q = il // 10, il % 10
            u0[32 * strip + q] = uni[r, :, 0]
            u1[32 * strip + q] = uni[r, :, 1]
        pack = np.concatenate(
            pack_common + [m1, m2, rsel.transpose(1, 0, 2).reshape(128, 3 * RPC),
                           u0, u1], axis=1)
        in_maps.append(dict(
            xc=xck, fcwh=fcwkh, fcwr=fcwkr, s1c=s1c, pack=pack))
    return in_maps


_cached_nc = None
last_exec_ns = None
last_results = None


def kernel(**inputs):
    global _cached_nc, last_exec_ns, last_results
    in_maps = _host_prep(inputs)
    if _cached_nc is None:
        _cached_nc = _build_program()
    trace = bool(int(os.environ.get("KERNEL_TRACE", "1")))
    res = bass_utils.run_bass_kernel_spmd(
        _cached_nc, in_maps, core_ids=list(range(NCORES)), trace=trace,
        tmpdir=os.environ.get("KERNEL_TRACE_DIR") or None)
    last_exec_ns = res.exec_time_ns
    last_results = res
    adj = np.concatenate([res.results[k]["adj"] for k in range(NCORES)],
                         axis=0)[:N]
    adj = np.ascontiguousarray(adj, dtype=np.float32)
    idx = np.arange(N)
    adj[idx, idx] = 0.0
    return adj
